# revision 6
# baseline (speedup 1.0000x reference)
"""CPM3 attention kernel for 8 trn2 NeuronCores.

Sharding: tensor-parallel over heads (2 heads/core x both batches).
Device computes per-core partial outputs (Wo row-sharded); host sums.
"""

import sys

sys.path.insert(0, "/opt/trn_rl_repo")

import numpy as np

import concourse.bass as bass
import concourse.bacc as bacc
import concourse.tile as tile
import concourse.mybir as mybir
from concourse.bass_utils import run_bass_kernel_spmd

B, L, D, H, DH = 2, 2048, 1024, 16, 64
N_CORES = 8
HPC = H // N_CORES  # heads per core = 2
D2H = HPC * DH  # 128
QTS = 512  # q tile size
QN = L // QTS  # 4
KP = 128  # k partition tile
KN = L // KP  # 16
KTG = 4  # k tiles per DMA group
KGN = KN // KTG  # 4
DC = D // 128  # 8 contraction chunks
HVW = 2 * (DH + 1)  # 130: hv_aug columns per k-tile (2 heads x (64+ones))

F32 = mybir.dt.float32
F32R = mybir.dt.float32r
U8 = mybir.dt.uint8
MM_NEG = 1.0e9  # additive mask magnitude

_CACHE: dict = {}


def _build():
    if "nc" in _CACHE:
        return _CACHE["nc"]
    nc = bacc.Bacc("TRN2", target_bir_lowering=False, debug=False, num_devices=N_CORES)

    qT = nc.dram_tensor("qT", [B, DC, 128, L], F32R, kind="ExternalInput").ap()
    kvT = nc.dram_tensor("kvT", [B, DC, 128, L], F32R, kind="ExternalInput").ap()
    wq = nc.dram_tensor("wq", [128, DC, 128], F32R, kind="ExternalInput").ap()
    wk = nc.dram_tensor("wk", [128, DC, 128], F32R, kind="ExternalInput").ap()
    wv = nc.dram_tensor("wv", [128, DC, 128], F32R, kind="ExternalInput").ap()
    wo = nc.dram_tensor("wo", [128, D], F32R, kind="ExternalInput").ap()
    pb = nc.dram_tensor(
        "pb", [QN, KGN, 128, KTG, HPC, QTS], F32R, kind="ExternalInput"
    ).ap()
    mk = nc.dram_tensor("mk", [QN, 128, B, KN, QTS], U8, kind="ExternalInput").ap()
    ident = nc.dram_tensor("ident", [128, 128], F32R, kind="ExternalInput").ap()
    out = nc.dram_tensor("out", [B, L, D], F32, kind="ExternalOutput").ap()

    with tile.TileContext(nc) as tc:
        with (
            tc.tile_pool(name="const", bufs=1) as constp,
            tc.tile_pool(name="hq", bufs=2) as hqp,
            tc.tile_pool(name="hk", bufs=2) as hkp,
            tc.tile_pool(name="hv", bufs=2) as hvp,
            tc.tile_pool(name="big", bufs=3) as bigp,
            tc.tile_pool(name="mask", bufs=2) as mkp,
            tc.tile_pool(name="ma", bufs=4) as map_,
            tc.tile_pool(name="pt", bufs=4) as ptp,
            tc.tile_pool(name="ctxn", bufs=2) as ctxnp,
            tc.tile_pool(name="rc", bufs=2) as rcp,
            tc.tile_pool(name="outb", bufs=3) as outp,
            tc.tile_pool(name="psum", bufs=8, space=bass.MemorySpace.PSUM) as psp,
        ):
            # ---- constants ----
            ident_t = constp.tile([128, 128], F32R, tag="ident")
            nc.sync.dma_start(ident_t[:], ident[:])
            wq_t = constp.tile([128, DC, 128], F32R, tag="wq")
            nc.sync.dma_start(wq_t[:], wq[:])
            wk_t = constp.tile([128, DC, 128], F32R, tag="wk")
            nc.sync.dma_start(wk_t[:], wk[:])
            wv_t = constp.tile([128, DC, 128], F32R, tag="wv")
            nc.sync.dma_start(wv_t[:], wv[:])
            wo_t = constp.tile([128, D], F32R, tag="wo")
            nc.sync.dma_start(wo_t[:], wo[:])
            ones1 = constp.tile([1, 64], F32, tag="ones1")
            nc.gpsimd.memset(ones1[:], 1.0)

            # ---- prologue: projections ----
            hq_sb, hk_sb, hv_sb = {}, {}, {}
            for b in range(B):
                hq_sb[b] = hqp.tile([128, L], F32R, tag="hq", name=f"hq_sb{b}")
                # hqT accumulation: 4 live psum banks across dc loop
                hq_ps = [psp.tile([128, QTS], F32, tag="bank", name=f"hq_ps{b}_{i}") for i in range(QN)]
                for dc in range(DC):
                    qc = bigp.tile([128, L], F32R, tag="big")
                    nc.sync.dma_start(qc[:], qT[b, dc])
                    for qt in range(QN):
                        nc.tensor.matmul(
                            hq_ps[qt][:],
                            wq_t[:, dc, :],
                            qc[:, qt * QTS : (qt + 1) * QTS],
                            start=(dc == 0),
                            stop=(dc == DC - 1),
                        )
                for qt in range(QN):
                    nc.vector.tensor_copy(
                        hq_sb[b][:, qt * QTS : (qt + 1) * QTS], hq_ps[qt][:]
                    )

                hk_sb[b] = hkp.tile([128, L], F32R, tag="hk", name=f"hk_sb{b}")
                hvT = bigp.tile([128, L], F32R, tag="big")
                hk_ps = [psp.tile([128, QTS], F32, tag="bank", name=f"hk_ps{b}_{i}") for i in range(QN)]
                hv_ps = [psp.tile([128, QTS], F32, tag="bank", name=f"hv_ps{b}_{i}") for i in range(QN)]
                for dc in range(DC):
                    kc = bigp.tile([128, L], F32R, tag="big")
                    nc.sync.dma_start(kc[:], kvT[b, dc])
                    for qt in range(QN):
                        nc.tensor.matmul(
                            hk_ps[qt][:],
                            wk_t[:, dc, :],
                            kc[:, qt * QTS : (qt + 1) * QTS],
                            start=(dc == 0),
                            stop=(dc == DC - 1),
                        )
                        nc.tensor.matmul(
                            hv_ps[qt][:],
                            wv_t[:, dc, :],
                            kc[:, qt * QTS : (qt + 1) * QTS],
                            start=(dc == 0),
                            stop=(dc == DC - 1),
                        )
                for qt in range(QN):
                    nc.vector.tensor_copy(
                        hk_sb[b][:, qt * QTS : (qt + 1) * QTS], hk_ps[qt][:]
                    )
                    nc.vector.tensor_copy(
                        hvT[:, qt * QTS : (qt + 1) * QTS], hv_ps[qt][:]
                    )

                # hv_aug: transpose hvT per k-tile; ones cols prefilled
                hv_sb[b] = hvp.tile([128, KN * HVW], F32R, tag="hv", name=f"hv_sb{b}")
                nc.gpsimd.memset(hv_sb[b][:].bitcast(mybir.dt.uint32), 0x3F800000)
                for kt in range(KN):
                    tp = psp.tile([128, 128], F32R, tag="bank")
                    nc.tensor.transpose(
                        tp[:], hvT[:, kt * KP : (kt + 1) * KP], ident_t[:]
                    )
                    o = kt * HVW
                    nc.vector.tensor_copy(hv_sb[b][:, o : o + DH], tp[:, 0:DH])
                    nc.vector.tensor_copy(
                        hv_sb[b][:, o + DH + 1 : o + 2 * DH + 1], tp[:, DH:128]
                    )

            # ---- main loop ----
            for qt in range(QN):
                mk_t = mkp.tile([128, B, KN, QTS], U8, tag="mask")
                nc.sync.dma_start(mk_t[:], mk[qt])
                ctx_ps = {}
                for b in range(B):
                    for h in range(HPC):
                        ctx_ps[(b, h)] = psp.tile([DH + 1, QTS], F32, tag="bank", name=f"ctx_ps{b}_{h}_{qt}")
                for kg in range(KGN):
                    pb_t = bigp.tile([128, KTG, HPC, QTS], F32R, tag="big")
                    nc.sync.dma_start(pb_t[:], pb[qt, kg])
                    for ki in range(KTG):
                        kt = kg * KTG + ki
                        ma_t = {}
                        for b in range(B):
                            ma_t[b] = map_.tile([128, QTS], F32R, tag="ma", name=f"ma_t{b}_{kt}")
                            nc.vector.tensor_scalar(
                                ma_t[b][:],
                                mk_t[:, b, kt, :],
                                MM_NEG,
                                -MM_NEG,
                                mybir.AluOpType.mult,
                                mybir.AluOpType.add,
                            )
                        for b in range(B):
                            for h in range(HPC):
                                sc = psp.tile([128, QTS], F32, tag="bank")
                                nc.tensor.matmul(
                                    sc[:],
                                    ident_t[:],
                                    pb_t[:, ki, h, :],
                                    start=True,
                                    stop=False,
                                )
                                nc.tensor.matmul(
                                    sc[:], ident_t[:], ma_t[b][:], start=False, stop=False
                                )
                                nc.tensor.matmul(
                                    sc[:],
                                    hk_sb[b][h * DH : (h + 1) * DH, kt * KP : (kt + 1) * KP],
                                    hq_sb[b][h * DH : (h + 1) * DH, qt * QTS : (qt + 1) * QTS],
                                    start=False,
                                    stop=True,
                                )
                                p_t = ptp.tile([128, QTS], F32R, tag="pt")
                                nc.scalar.activation(
                                    p_t[:], sc[:], mybir.ActivationFunctionType.Exp
                                )
                                o = kt * HVW + h * (DH + 1)
                                nc.tensor.matmul(
                                    ctx_ps[(b, h)][:],
                                    hv_sb[b][:, o : o + DH + 1],
                                    p_t[:],
                                    start=(kt == 0),
                                    stop=(kt == KN - 1),
                                )
                # normalize + output projection
                for b in range(B):
                    ctxn = ctxnp.tile([128, QTS], F32R, tag="ctxn")
                    for h in range(HPC):
                        rc = rcp.tile([1, QTS], F32, tag="rc")
                        nc.vector.reciprocal(rc[:], ctx_ps[(b, h)][DH : DH + 1, :])
                        bc = psp.tile([64, QTS], F32, tag="bank")
                        nc.tensor.matmul(bc[:], ones1[:], rc[:], start=True, stop=True)
                        bc_sb = rcp.tile([64, QTS], F32, tag="bcsb", name=f"bc_sb{b}_{h}")
                        nc.vector.tensor_copy(bc_sb[:], bc[:])
                        nc.vector.tensor_tensor(
                            ctxn[h * DH : (h + 1) * DH, :],
                            ctx_ps[(b, h)][0:DH, :],
                            bc_sb[:],
                            mybir.AluOpType.mult,
                        )
                    for qs in range(QN):
                        ob = outp.tile([128, D], F32, tag="outb")
                        for oh in range(2):
                            op_ps = psp.tile([128, QTS], F32, tag="bank")
                            nc.tensor.matmul(
                                op_ps[:],
                                ctxn[:, qs * 128 : (qs + 1) * 128],
                                wo_t[:, oh * QTS : (oh + 1) * QTS],
                                start=True,
                                stop=True,
                            )
                            nc.vector.tensor_copy(
                                ob[:, oh * QTS : (oh + 1) * QTS], op_ps[:]
                            )
                        r0 = qt * QTS + qs * 128
                        nc.sync.dma_start(out[b, r0 : r0 + 128, :], ob[:])

    nc.compile()
    _CACHE["nc"] = nc
    return nc


def _prep_core(core, query, key_value, mask, position_bias, Wq, Wk, Wv, Wo, shared):
    """Per-core input map. `shared` holds core-independent packed arrays."""
    h0 = core * HPC
    rows = slice(h0 * DH, (h0 + HPC) * DH)
    # weight chunks packed [dp, dc, d2h] from W[rows].T [D, 128]
    def packw(w, scale=1.0):
        wt = np.ascontiguousarray(
            (w[rows].T * scale).reshape(DC, 128, 128).transpose(1, 0, 2),
            dtype=np.float32,
        )
        return wt

    pbc = position_bias[h0 : h0 + HPC]  # [2, q, k]
    # -> [qt, kg, kp, ki, h, qf]
    pbp = np.ascontiguousarray(
        pbc.reshape(HPC, QN, QTS, KGN, KTG, 128).transpose(1, 3, 5, 4, 0, 2),
        dtype=np.float32,
    )
    m = {
        "qT": shared["qT"],
        "kvT": shared["kvT"],
        "mk": shared["mk"],
        "ident": shared["ident"],
        "wq": packw(Wq, 1.0 / np.sqrt(DH)),
        "wk": packw(Wk),
        "wv": packw(Wv),
        "wo": np.ascontiguousarray(Wo[:, rows].T, dtype=np.float32),
        "pb": pbp,
    }
    return m


def _prep_shared(query, key_value, mask):
    qTp = np.ascontiguousarray(
        query.reshape(B, L, DC, 128).transpose(0, 2, 3, 1), dtype=np.float32
    )
    kvTp = np.ascontiguousarray(
        key_value.reshape(B, L, DC, 128).transpose(0, 2, 3, 1), dtype=np.float32
    )
    mku = mask.astype(np.uint8)
    # [b, q, k] -> [qt, kp, b, kt, qf]
    mkp = np.ascontiguousarray(
        mku.reshape(B, QN, QTS, KN, 128).transpose(1, 4, 0, 3, 2)
    )
    return {
        "qT": qTp,
        "kvT": kvTp,
        "mk": mkp,
        "ident": np.eye(128, dtype=np.float32),
    }


def kernel(query, key_value, mask, position_bias, Wq, Wk, Wv, Wo, _trace=False):
    query = np.asarray(query, dtype=np.float32)
    key_value = np.asarray(key_value, dtype=np.float32)
    mask = np.asarray(mask)
    position_bias = np.asarray(position_bias, dtype=np.float32)
    Wq = np.asarray(Wq, dtype=np.float32)
    Wk = np.asarray(Wk, dtype=np.float32)
    Wv = np.asarray(Wv, dtype=np.float32)
    Wo = np.asarray(Wo, dtype=np.float32)

    nc = _build()
    shared = _prep_shared(query, key_value, mask)
    in_maps = [
        _prep_core(c, query, key_value, mask, position_bias, Wq, Wk, Wv, Wo, shared)
        for c in range(N_CORES)
    ]
    res = run_bass_kernel_spmd(
        nc, in_maps, list(range(N_CORES)), trace=_trace
    )
    _CACHE["last_result"] = res
    acc = res.results[0]["out"].astype(np.float64)
    for c in range(1, N_CORES):
        acc += res.results[c]["out"]
    return acc.astype(np.float32)


# revision 13
# speedup vs baseline: 1.1529x; 1.1529x over previous
"""CPM3 attention kernel for 8 trn2 NeuronCores.

Sharding: tensor-parallel over heads (2 heads/core x both batches).
Device computes per-core partial outputs (Wo row-sharded); host sums.
"""

import sys

sys.path.insert(0, "/opt/trn_rl_repo")

import numpy as np
import ml_dtypes

import concourse.bass as bass
import concourse.bacc as bacc
import concourse.tile as tile
import concourse.mybir as mybir
from concourse.bass_utils import run_bass_kernel_spmd

B, L, D, H, DH = 2, 2048, 1024, 16, 64
N_CORES = 8
HPC = H // N_CORES  # heads per core = 2
D2H = HPC * DH  # 128
QTS = 512  # q tile size
QN = L // QTS  # 4
KP = 128  # k partition tile
KN = L // KP  # 16
KTG = 4  # k tiles per DMA group
KGN = KN // KTG  # 4
DC = D // 128  # 8 contraction chunks
HVW = 2 * (DH + 1)  # 130: hv_aug columns per k-tile (2 heads x (64+ones))

F32 = mybir.dt.float32
F32R = mybir.dt.float32r
U8 = mybir.dt.uint8
MM_NEG = 1.0e9  # additive mask magnitude

_CACHE: dict = {}


def _build():
    if "nc" in _CACHE:
        return _CACHE["nc"]
    nc = bacc.Bacc("TRN2", target_bir_lowering=False, debug=False, num_devices=N_CORES)

    qT = nc.dram_tensor("qT", [B, DC, 128, L], F32R, kind="ExternalInput").ap()
    kvT = nc.dram_tensor("kvT", [B, DC, 128, L], F32R, kind="ExternalInput").ap()
    wq = nc.dram_tensor("wq", [128, DC, 128], F32R, kind="ExternalInput").ap()
    wk = nc.dram_tensor("wk", [128, DC, 128], F32R, kind="ExternalInput").ap()
    wv = nc.dram_tensor("wv", [128, DC, 128], F32R, kind="ExternalInput").ap()
    wo = nc.dram_tensor("wo", [128, D], F32R, kind="ExternalInput").ap()
    pb = nc.dram_tensor(
        "pb", [QN, KGN, 128, KTG, HPC, QTS], F32R, kind="ExternalInput"
    ).ap()
    mk = nc.dram_tensor("mk", [QN, 128, B, KN, QTS], U8, kind="ExternalInput").ap()
    ident = nc.dram_tensor("ident", [128, 128], F32R, kind="ExternalInput").ap()
    indh = nc.dram_tensor("indh", [1, 256], F32, kind="ExternalInput").ap()
    out = nc.dram_tensor("out", [B, L, D], F32, kind="ExternalOutput").ap()

    with tile.TileContext(nc) as tc:
        with (
            tc.tile_pool(name="const", bufs=1) as constp,
            tc.tile_pool(name="hq", bufs=2) as hqp,
            tc.tile_pool(name="hk", bufs=2) as hkp,
            tc.tile_pool(name="hv", bufs=2) as hvp,
            tc.tile_pool(name="big", bufs=3) as bigp,
            tc.tile_pool(name="mask", bufs=2) as mkp,
            tc.tile_pool(name="ma", bufs=4) as map_,
            tc.tile_pool(name="pt", bufs=4) as ptp,
            tc.tile_pool(name="ctxn", bufs=2) as ctxnp,
            tc.tile_pool(name="rc", bufs=2) as rcp,
            tc.tile_pool(name="outb", bufs=3) as outp,
            tc.tile_pool(name="psum", bufs=8, space=bass.MemorySpace.PSUM) as psp,
        ):
            # ---- constants ----
            ident_t = constp.tile([128, 128], F32R, tag="ident")
            nc.sync.dma_start(ident_t[:], ident[:])
            indh_t = constp.tile([1, 256], F32, tag="indh")
            nc.sync.dma_start(indh_t[:], indh[:])
            wq_t = constp.tile([128, DC, 128], F32R, tag="wq")
            nc.sync.dma_start(wq_t[:], wq[:])
            wk_t = constp.tile([128, DC, 128], F32R, tag="wk")
            nc.sync.dma_start(wk_t[:], wk[:])
            wv_t = constp.tile([128, DC, 128], F32R, tag="wv")
            nc.sync.dma_start(wv_t[:], wv[:])
            wo_t = constp.tile([128, D], F32R, tag="wo")
            nc.sync.dma_start(wo_t[:], wo[:])

            # ---- prologue: projections ----
            hq_sb, hk_sb, hv_sb = {}, {}, {}
            for b in range(B):
                hq_sb[b] = hqp.tile([128, L], F32R, tag="hq", name=f"hq_sb{b}")
                # hqT accumulation: 4 live psum banks across dc loop
                hq_ps = [psp.tile([128, QTS], F32, tag="bank", name=f"hq_ps{b}_{i}") for i in range(QN)]
                for dc in range(DC):
                    qc = bigp.tile([128, L], F32R, tag="big")
                    nc.sync.dma_start(qc[:], qT[b, dc])
                    for qt in range(QN):
                        nc.tensor.matmul(
                            hq_ps[qt][:],
                            wq_t[:, dc, :],
                            qc[:, qt * QTS : (qt + 1) * QTS],
                            start=(dc == 0),
                            stop=(dc == DC - 1),
                        )
                for qt in range(QN):
                    nc.vector.tensor_copy(
                        hq_sb[b][:, qt * QTS : (qt + 1) * QTS], hq_ps[qt][:]
                    )

                hk_sb[b] = hkp.tile([128, L], F32R, tag="hk", name=f"hk_sb{b}")
                hvT = bigp.tile([128, L], F32R, tag="big")
                hk_ps = [psp.tile([128, QTS], F32, tag="bank", name=f"hk_ps{b}_{i}") for i in range(QN)]
                hv_ps = [psp.tile([128, QTS], F32, tag="bank", name=f"hv_ps{b}_{i}") for i in range(QN)]
                for dc in range(DC):
                    kc = bigp.tile([128, L], F32R, tag="big")
                    nc.sync.dma_start(kc[:], kvT[b, dc])
                    for qt in range(QN):
                        nc.tensor.matmul(
                            hk_ps[qt][:],
                            wk_t[:, dc, :],
                            kc[:, qt * QTS : (qt + 1) * QTS],
                            start=(dc == 0),
                            stop=(dc == DC - 1),
                        )
                        nc.tensor.matmul(
                            hv_ps[qt][:],
                            wv_t[:, dc, :],
                            kc[:, qt * QTS : (qt + 1) * QTS],
                            start=(dc == 0),
                            stop=(dc == DC - 1),
                        )
                for qt in range(QN):
                    nc.vector.tensor_copy(
                        hk_sb[b][:, qt * QTS : (qt + 1) * QTS], hk_ps[qt][:]
                    )
                    nc.vector.tensor_copy(
                        hvT[:, qt * QTS : (qt + 1) * QTS], hv_ps[qt][:]
                    )

                # hv_aug: transpose hvT per k-tile; ones cols prefilled
                hv_sb[b] = hvp.tile([128, KN * HVW], F32R, tag="hv", name=f"hv_sb{b}")
                nc.gpsimd.memset(hv_sb[b][:].bitcast(mybir.dt.uint32), 0x3F800000)
                for kt in range(KN):
                    tp = psp.tile([128, 128], F32R, tag="bank")
                    nc.tensor.transpose(
                        tp[:], hvT[:, kt * KP : (kt + 1) * KP], ident_t[:]
                    )
                    o = kt * HVW
                    nc.vector.tensor_copy(hv_sb[b][:, o : o + DH], tp[:, 0:DH])
                    nc.vector.tensor_copy(
                        hv_sb[b][:, o + DH + 1 : o + 2 * DH + 1], tp[:, DH:128]
                    )

            # ---- main loop ----
            for qt in range(QN):
                mk_t = mkp.tile([128, B, KN, QTS], U8, tag="mask")
                nc.sync.dma_start(mk_t[:], mk[qt])
                ctx_ps = {}
                for b in range(B):
                    for h in range(HPC):
                        ctx_ps[(b, h)] = psp.tile([DH + 1, QTS], F32, tag="bank", name=f"ctx_ps{b}_{h}_{qt}")
                for kg in range(KGN):
                    pb_t = bigp.tile([128, KTG, HPC, QTS], F32R, tag="big")
                    nc.sync.dma_start(pb_t[:], pb[qt, kg])
                    for ki in range(KTG):
                        kt = kg * KTG + ki
                        ma_t = {}
                        for b in range(B):
                            ma_t[b] = map_.tile([128, QTS], F32R, tag="ma", name=f"ma_t{b}_{kt}")
                            nc.vector.tensor_scalar(
                                ma_t[b][:],
                                mk_t[:, b, kt, :],
                                MM_NEG,
                                -MM_NEG,
                                mybir.AluOpType.mult,
                                mybir.AluOpType.add,
                            )
                        for b in range(B):
                            sc = {}
                            for h in range(HPC):
                                sc[h] = psp.tile(
                                    [128, QTS], F32, tag="bank", name=f"sc{b}_{h}_{kt}"
                                )
                                nc.tensor.matmul(
                                    sc[h][:],
                                    ident_t[:],
                                    pb_t[:, ki, h, :],
                                    start=True,
                                    stop=False,
                                )
                                nc.tensor.matmul(
                                    sc[h][:],
                                    ident_t[:],
                                    ma_t[b][:],
                                    start=False,
                                    stop=False,
                                )
                            # QK pair: h0 rows 0-63, h1 rows 64-127 run concurrently
                            for h in range(HPC):
                                nc.tensor.matmul(
                                    sc[h][:],
                                    hk_sb[b][h * DH : (h + 1) * DH, kt * KP : (kt + 1) * KP],
                                    hq_sb[b][h * DH : (h + 1) * DH, qt * QTS : (qt + 1) * QTS],
                                    start=False,
                                    stop=True,
                                )
                            for h in range(HPC):
                                p_t = ptp.tile(
                                    [128, QTS], F32R, tag="pt", name=f"p_t{b}_{h}_{kt}"
                                )
                                nc.scalar.activation(
                                    p_t[:], sc[h][:], mybir.ActivationFunctionType.Exp
                                )
                                o = kt * HVW + h * (DH + 1)
                                nc.tensor.matmul(
                                    ctx_ps[(b, h)][:],
                                    hv_sb[b][:, o : o + DH + 1],
                                    p_t[:],
                                    start=(kt == 0),
                                    stop=(kt == KN - 1),
                                )
                # normalize + output projection
                for b in range(B):
                    ctxn = ctxnp.tile([128, QTS], F32R, tag="ctxn")
                    bc = psp.tile([128, QTS], F32, tag="bank")
                    for h in range(HPC):
                        dsb = rcp.tile([1, QTS], F32, tag="dsb", name=f"dsb{b}_{h}")
                        nc.vector.tensor_copy(dsb[:], ctx_ps[(b, h)][DH : DH + 1, :])
                        rcf = rcp.tile([1, QTS], F32, tag="rcf", name=f"rcf{b}_{h}")
                        nc.vector.reciprocal_approx_fast(rcf[:], dsb[:])
                        nc.tensor.matmul(
                            bc[:],
                            indh_t[:, h * 128 : (h + 1) * 128],
                            rcf[:],
                            start=(h == 0),
                            stop=(h == HPC - 1),
                        )
                    bc_sb = rcp.tile([128, QTS], F32, tag="bcsb", name=f"bc_sb{b}")
                    nc.vector.tensor_copy(bc_sb[:], bc[:])
                    for h in range(HPC):
                        nc.vector.tensor_tensor(
                            ctxn[h * DH : (h + 1) * DH, :],
                            ctx_ps[(b, h)][0:DH, :],
                            bc_sb[h * DH : (h + 1) * DH, :],
                            mybir.AluOpType.mult,
                        )
                    for qs in range(QN):
                        ob = outp.tile([128, D], F32, tag="outb")
                        for oh in range(2):
                            op_ps = psp.tile([128, QTS], F32, tag="bank")
                            nc.tensor.matmul(
                                op_ps[:],
                                ctxn[:, qs * 128 : (qs + 1) * 128],
                                wo_t[:, oh * QTS : (oh + 1) * QTS],
                                start=True,
                                stop=True,
                            )
                            nc.vector.tensor_copy(
                                ob[:, oh * QTS : (oh + 1) * QTS], op_ps[:]
                            )
                        r0 = qt * QTS + qs * 128
                        nc.sync.dma_start(out[b, r0 : r0 + 128, :], ob[:])

    nc.compile()
    _CACHE["nc"] = nc
    return nc


def _prep_core(core, query, key_value, mask, position_bias, Wq, Wk, Wv, Wo, shared):
    """Per-core input map. `shared` holds core-independent packed arrays."""
    h0 = core * HPC
    rows = slice(h0 * DH, (h0 + HPC) * DH)
    # weight chunks packed [dp, dc, d2h] from W[rows].T [D, 128]
    def packw(w, scale=1.0):
        wt = np.ascontiguousarray(
            (w[rows].T * scale).reshape(DC, 128, 128).transpose(1, 0, 2),
            dtype=np.float32,
        )
        return wt

    pbc = position_bias[h0 : h0 + HPC]  # [2, q, k]
    # -> [qt, kg, kp, ki, h, qf]
    pbp = np.ascontiguousarray(
        pbc.reshape(HPC, QN, QTS, KGN, KTG, 128).transpose(1, 3, 5, 4, 0, 2),
        dtype=np.float32,
    )
    m = {
        "qT": shared["qT"],
        "kvT": shared["kvT"],
        "mk": shared["mk"],
        "ident": shared["ident"],
        "indh": shared["indh"],
        "wq": packw(Wq, 1.0 / np.sqrt(DH)),
        "wk": packw(Wk),
        "wv": packw(Wv),
        "wo": np.ascontiguousarray(Wo[:, rows].T, dtype=np.float32),
        "pb": pbp,
    }
    return m


def _prep_shared(query, key_value, mask):
    qTp = np.ascontiguousarray(
        query.reshape(B, L, DC, 128).transpose(0, 2, 3, 1), dtype=np.float32
    )
    kvTp = np.ascontiguousarray(
        key_value.reshape(B, L, DC, 128).transpose(0, 2, 3, 1), dtype=np.float32
    )
    mku = mask.astype(np.uint8)
    # [b, q, k] -> [qt, kp, b, kt, qf]
    mkp = np.ascontiguousarray(
        mku.reshape(B, QN, QTS, KN, 128).transpose(1, 4, 0, 3, 2)
    )
    return {
        "qT": qTp,
        "kvT": kvTp,
        "mk": mkp,
        "ident": np.eye(128, dtype=np.float32),
        "indh": np.ascontiguousarray(
            np.concatenate(
                [
                    np.where(np.arange(128) < 64, 1.0, 0.0),
                    np.where(np.arange(128) >= 64, 1.0, 0.0),
                ]
            ).astype(np.float32)[None, :]
        ),
    }


def kernel(query, key_value, mask, position_bias, Wq, Wk, Wv, Wo, _trace=False):
    query = np.asarray(query, dtype=np.float32)
    key_value = np.asarray(key_value, dtype=np.float32)
    mask = np.asarray(mask)
    position_bias = np.asarray(position_bias, dtype=np.float32)
    Wq = np.asarray(Wq, dtype=np.float32)
    Wk = np.asarray(Wk, dtype=np.float32)
    Wv = np.asarray(Wv, dtype=np.float32)
    Wo = np.asarray(Wo, dtype=np.float32)

    nc = _build()
    shared = _prep_shared(query, key_value, mask)
    in_maps = [
        _prep_core(c, query, key_value, mask, position_bias, Wq, Wk, Wv, Wo, shared)
        for c in range(N_CORES)
    ]
    res = run_bass_kernel_spmd(
        nc, in_maps, list(range(N_CORES)), trace=_trace
    )
    _CACHE["last_result"] = res
    acc = res.results[0]["out"].astype(np.float64)
    for c in range(1, N_CORES):
        acc += res.results[c]["out"]
    return acc.astype(np.float32)


# revision 16
# speedup vs baseline: 1.1607x; 1.0068x over previous
"""CPM3 attention kernel for 8 trn2 NeuronCores.

Sharding: tensor-parallel over heads (2 heads/core x both batches).
Device computes per-core partial outputs (Wo row-sharded); host sums.
"""

import sys

sys.path.insert(0, "/opt/trn_rl_repo")

import numpy as np
import ml_dtypes

import concourse.bass as bass
import concourse.bacc as bacc
import concourse.tile as tile
import concourse.mybir as mybir
from concourse.bass_utils import run_bass_kernel_spmd

B, L, D, H, DH = 2, 2048, 1024, 16, 64
N_CORES = 8
HPC = H // N_CORES  # heads per core = 2
D2H = HPC * DH  # 128
QTS = 512  # q tile size
QN = L // QTS  # 4
KP = 128  # k partition tile
KN = L // KP  # 16
KTG = 4  # k tiles per DMA group
KGN = KN // KTG  # 4
DC = D // 128  # 8 contraction chunks
HVW = 2 * (DH + 1)  # 130: hv_aug columns per k-tile (2 heads x (64+ones))

F32 = mybir.dt.float32
F32R = mybir.dt.float32r
U8 = mybir.dt.uint8
BF16 = mybir.dt.bfloat16
MM_NEG = 1.0e9  # additive mask magnitude

_CACHE: dict = {}


def _build():
    if "nc" in _CACHE:
        return _CACHE["nc"]
    nc = bacc.Bacc("TRN2", target_bir_lowering=False, debug=False, num_devices=N_CORES)

    qT = nc.dram_tensor("qT", [B, DC, 128, L], F32R, kind="ExternalInput").ap()
    kvT = nc.dram_tensor("kvT", [B, DC, 128, L], F32R, kind="ExternalInput").ap()
    wq = nc.dram_tensor("wq", [128, DC, 128], F32R, kind="ExternalInput").ap()
    wk = nc.dram_tensor("wk", [128, DC, 128], F32R, kind="ExternalInput").ap()
    wv = nc.dram_tensor("wv", [128, DC, 128], F32R, kind="ExternalInput").ap()
    wo = nc.dram_tensor("wo", [128, D], F32R, kind="ExternalInput").ap()
    pb = nc.dram_tensor(
        "pb", [QN, KGN, 128, KTG, HPC, QTS], F32R, kind="ExternalInput"
    ).ap()
    mk = nc.dram_tensor("mk", [QN, 128, B, KN, QTS], U8, kind="ExternalInput").ap()
    ident = nc.dram_tensor("ident", [128, 128], F32R, kind="ExternalInput").ap()
    identb = nc.dram_tensor("identb", [128, 128], mybir.dt.bfloat16, kind="ExternalInput").ap()
    indh = nc.dram_tensor("indh", [1, 256], F32, kind="ExternalInput").ap()
    out = nc.dram_tensor("out", [B, L, D], F32, kind="ExternalOutput").ap()

    with tile.TileContext(nc) as tc:
        with (
            tc.tile_pool(name="const", bufs=1) as constp,
            tc.tile_pool(name="hq", bufs=2) as hqp,
            tc.tile_pool(name="hk", bufs=2) as hkp,
            tc.tile_pool(name="hv", bufs=2) as hvp,
            tc.tile_pool(name="big", bufs=3) as bigp,
            tc.tile_pool(name="mask", bufs=2) as mkp,
            tc.tile_pool(name="ma", bufs=4) as map_,
            tc.tile_pool(name="pt", bufs=3) as ptp,
            tc.tile_pool(name="ctxn", bufs=2) as ctxnp,
            tc.tile_pool(name="rc", bufs=2) as rcp,
            tc.tile_pool(name="outb", bufs=3) as outp,
            tc.tile_pool(name="psum", bufs=4, space=bass.MemorySpace.PSUM) as psp,
            tc.tile_pool(name="psw", bufs=2, space=bass.MemorySpace.PSUM) as psw,
        ):
            # ---- constants ----
            ident_t = constp.tile([128, 128], F32R, tag="ident")
            nc.sync.dma_start(ident_t[:], ident[:])
            identb_t = constp.tile([128, 128], BF16, tag="identb")
            nc.sync.dma_start(identb_t[:], identb[:])
            indh_t = constp.tile([1, 256], F32, tag="indh")
            nc.sync.dma_start(indh_t[:], indh[:])
            wq_t = constp.tile([128, DC, 128], F32R, tag="wq")
            nc.sync.dma_start(wq_t[:], wq[:])
            wk_t = constp.tile([128, DC, 128], F32R, tag="wk")
            nc.sync.dma_start(wk_t[:], wk[:])
            wv_t = constp.tile([128, DC, 128], F32R, tag="wv")
            nc.sync.dma_start(wv_t[:], wv[:])
            wo_t = constp.tile([128, D], F32R, tag="wo")
            nc.sync.dma_start(wo_t[:], wo[:])

            # ---- prologue: projections ----
            hq_sb, hk_sb, hv_sb = {}, {}, {}
            for b in range(B):
                hq_sb[b] = hqp.tile([128, L], F32R, tag="hq", name=f"hq_sb{b}")
                hq_ps = [psw.tile([128, 2 * QTS], F32, tag="w", name=f"hq_ps{b}_{i}") for i in range(2)]
                for dc in range(DC):
                    qc = bigp.tile([128, L], F32R, tag="big")
                    nc.sync.dma_start(qc[:], qT[b, dc])
                    for qt in range(QN):
                        nc.tensor.matmul(
                            hq_ps[qt // 2][:, (qt % 2) * QTS : (qt % 2 + 1) * QTS],
                            wq_t[:, dc, :],
                            qc[:, qt * QTS : (qt + 1) * QTS],
                            start=(dc == 0),
                            stop=(dc == DC - 1),
                        )
                for qt in range(QN):
                    nc.vector.tensor_copy(
                        hq_sb[b][:, qt * QTS : (qt + 1) * QTS],
                        hq_ps[qt // 2][:, (qt % 2) * QTS : (qt % 2 + 1) * QTS],
                    )

                hk_sb[b] = hkp.tile([128, L], F32R, tag="hk", name=f"hk_sb{b}")
                hvT = bigp.tile([128, L], F32R, tag="big")
                hk_ps = [psw.tile([128, 2 * QTS], F32, tag="w", name=f"hk_ps{b}_{i}") for i in range(2)]
                hv_ps = [psp.tile([128, QTS], F32, tag="bank", name=f"hv_ps{b}_{i}") for i in range(QN)]
                for dc in range(DC):
                    kc = bigp.tile([128, L], F32R, tag="big")
                    nc.sync.dma_start(kc[:], kvT[b, dc])
                    for qt in range(QN):
                        nc.tensor.matmul(
                            hk_ps[qt // 2][:, (qt % 2) * QTS : (qt % 2 + 1) * QTS],
                            wk_t[:, dc, :],
                            kc[:, qt * QTS : (qt + 1) * QTS],
                            start=(dc == 0),
                            stop=(dc == DC - 1),
                        )
                        nc.tensor.matmul(
                            hv_ps[qt][:],
                            wv_t[:, dc, :],
                            kc[:, qt * QTS : (qt + 1) * QTS],
                            start=(dc == 0),
                            stop=(dc == DC - 1),
                        )
                for qt in range(QN):
                    nc.vector.tensor_copy(
                        hk_sb[b][:, qt * QTS : (qt + 1) * QTS],
                        hk_ps[qt // 2][:, (qt % 2) * QTS : (qt % 2 + 1) * QTS],
                    )
                    nc.vector.tensor_copy(
                        hvT[:, qt * QTS : (qt + 1) * QTS], hv_ps[qt][:]
                    )

                # hv_aug: transpose hvT per k-tile; ones cols prefilled
                hv_sb[b] = hvp.tile([128, KN * HVW], F32R, tag="hv", name=f"hv_sb{b}")
                nc.gpsimd.memset(hv_sb[b][:].bitcast(mybir.dt.uint32), 0x3F800000)
                for kt in range(KN):
                    tp = psp.tile([128, 128], F32R, tag="bank")
                    nc.tensor.transpose(
                        tp[:], hvT[:, kt * KP : (kt + 1) * KP], ident_t[:]
                    )
                    o = kt * HVW
                    nc.vector.tensor_copy(hv_sb[b][:, o : o + DH], tp[:, 0:DH])
                    nc.vector.tensor_copy(
                        hv_sb[b][:, o + DH + 1 : o + 2 * DH + 1], tp[:, DH:128]
                    )

            # ---- main loop ----
            for qt in range(QN):
                mk_t = mkp.tile([128, B, KN, QTS], U8, tag="mask")
                nc.sync.dma_start(mk_t[:], mk[qt])
                ctx_ps = {}
                for b in range(B):
                    for h in range(HPC):
                        ctx_ps[(b, h)] = psp.tile(
                            [DH + 1, QTS], F32, tag="bank", name=f"ctx_ps{b}_{h}_{qt}"
                        )
                for kg in range(KGN):
                    pb_t = bigp.tile([128, KTG, HPC, QTS], F32R, tag="big")
                    nc.sync.dma_start(pb_t[:], pb[qt, kg])
                    for ki in range(KTG):
                        kt = kg * KTG + ki
                        # additive mask, duplicated for both heads (bf16 exact)
                        ma_t = {}
                        for b in range(B):
                            ma_t[b] = map_.tile(
                                [128, 2 * QTS], BF16, tag="ma", name=f"ma_t{b}_{kt}"
                            )
                            for h in range(HPC):
                                nc.vector.tensor_scalar(
                                    ma_t[b][:, h * QTS : (h + 1) * QTS],
                                    mk_t[:, b, kt, :],
                                    MM_NEG,
                                    -MM_NEG,
                                    mybir.AluOpType.mult,
                                    mybir.AluOpType.add,
                                )
                        sc = {}
                        for b in range(B):
                            sc[b] = psw.tile(
                                [128, 2 * QTS], F32, tag="w", name=f"sc{b}_{kt}"
                            )
                            for h in range(HPC):
                                nc.tensor.matmul(
                                    sc[b][:, h * QTS : (h + 1) * QTS],
                                    ident_t[:],
                                    pb_t[:, ki, h, :],
                                    start=True,
                                    stop=False,
                                )
                            for h in range(HPC):
                                nc.tensor.matmul(
                                    sc[b][:, h * QTS : (h + 1) * QTS],
                                    identb_t[:],
                                    ma_t[b][:, h * QTS : (h + 1) * QTS],
                                    start=False,
                                    stop=False,
                                )
                        # QK pairs: h0 rows 0-63, h1 rows 64-127 run concurrently
                        for b in range(B):
                            for h in range(HPC):
                                nc.tensor.matmul(
                                    sc[b][:, h * QTS : (h + 1) * QTS],
                                    hk_sb[b][h * DH : (h + 1) * DH, kt * KP : (kt + 1) * KP],
                                    hq_sb[b][h * DH : (h + 1) * DH, qt * QTS : (qt + 1) * QTS],
                                    start=False,
                                    stop=True,
                                )
                        for b in range(B):
                            p_t = ptp.tile(
                                [128, 2 * QTS], F32R, tag="pt", name=f"p_t{b}_{kt}"
                            )
                            nc.scalar.activation(
                                p_t[:], sc[b][:], mybir.ActivationFunctionType.Exp
                            )
                            for h in range(HPC):
                                o = kt * HVW + h * (DH + 1)
                                nc.tensor.matmul(
                                    ctx_ps[(b, h)][:],
                                    hv_sb[b][:, o : o + DH + 1],
                                    p_t[:, h * QTS : (h + 1) * QTS],
                                    start=(kt == 0),
                                    stop=(kt == KN - 1),
                                )
                # normalize + output projection
                for b in range(B):
                    ctxn = ctxnp.tile([128, QTS], F32R, tag="ctxn")
                    bcw = psw.tile([128, 2 * QTS], F32, tag="w", name=f"bcw{b}")
                    bc = bcw[:, 0:QTS]
                    for h in range(HPC):
                        dsb = rcp.tile([1, QTS], F32, tag="dsb", name=f"dsb{b}_{h}")
                        nc.vector.tensor_copy(dsb[:], ctx_ps[(b, h)][DH : DH + 1, :])
                        rcf = rcp.tile([1, QTS], F32, tag="rcf", name=f"rcf{b}_{h}")
                        nc.vector.reciprocal_approx_fast(rcf[:], dsb[:])
                        nc.tensor.matmul(
                            bc,
                            indh_t[:, h * 128 : (h + 1) * 128],
                            rcf[:],
                            start=(h == 0),
                            stop=(h == HPC - 1),
                        )
                    bc_sb = rcp.tile([128, QTS], F32, tag="bcsb", name=f"bc_sb{b}")
                    nc.vector.tensor_copy(bc_sb[:], bc)
                    for h in range(HPC):
                        nc.vector.tensor_tensor(
                            ctxn[h * DH : (h + 1) * DH, :],
                            ctx_ps[(b, h)][0:DH, :],
                            bc_sb[h * DH : (h + 1) * DH, :],
                            mybir.AluOpType.mult,
                        )
                    for qs in range(QN):
                        ob = outp.tile([128, D], F32, tag="outb")
                        op_ps = psw.tile([128, 2 * QTS], F32, tag="w", name=f"op{b}_{qs}")
                        for oh in range(2):
                            nc.tensor.matmul(
                                op_ps[:, oh * QTS : (oh + 1) * QTS],
                                ctxn[:, qs * 128 : (qs + 1) * 128],
                                wo_t[:, oh * QTS : (oh + 1) * QTS],
                                start=True,
                                stop=True,
                            )
                        nc.vector.tensor_copy(ob[:], op_ps[:])
                        r0 = qt * QTS + qs * 128
                        nc.sync.dma_start(out[b, r0 : r0 + 128, :], ob[:])

    nc.compile()
    _CACHE["nc"] = nc
    return nc


def _prep_core(core, query, key_value, mask, position_bias, Wq, Wk, Wv, Wo, shared):
    """Per-core input map. `shared` holds core-independent packed arrays."""
    h0 = core * HPC
    rows = slice(h0 * DH, (h0 + HPC) * DH)
    # weight chunks packed [dp, dc, d2h] from W[rows].T [D, 128]
    def packw(w, scale=1.0):
        wt = np.ascontiguousarray(
            (w[rows].T * scale).reshape(DC, 128, 128).transpose(1, 0, 2),
            dtype=np.float32,
        )
        return wt

    pbc = position_bias[h0 : h0 + HPC]  # [2, q, k]
    # -> [qt, kg, kp, ki, h, qf]
    pbp = np.ascontiguousarray(
        pbc.reshape(HPC, QN, QTS, KGN, KTG, 128).transpose(1, 3, 5, 4, 0, 2),
        dtype=np.float32,
    )
    m = {
        "qT": shared["qT"],
        "kvT": shared["kvT"],
        "mk": shared["mk"],
        "ident": shared["ident"],
        "identb": shared["identb"],
        "indh": shared["indh"],
        "wq": packw(Wq, 1.0 / np.sqrt(DH)),
        "wk": packw(Wk),
        "wv": packw(Wv),
        "wo": np.ascontiguousarray(Wo[:, rows].T, dtype=np.float32),
        "pb": pbp,
    }
    return m


def _prep_shared(query, key_value, mask):
    qTp = np.ascontiguousarray(
        query.reshape(B, L, DC, 128).transpose(0, 2, 3, 1), dtype=np.float32
    )
    kvTp = np.ascontiguousarray(
        key_value.reshape(B, L, DC, 128).transpose(0, 2, 3, 1), dtype=np.float32
    )
    mku = mask.astype(np.uint8)
    # [b, q, k] -> [qt, kp, b, kt, qf]
    mkp = np.ascontiguousarray(
        mku.reshape(B, QN, QTS, KN, 128).transpose(1, 4, 0, 3, 2)
    )
    return {
        "qT": qTp,
        "kvT": kvTp,
        "mk": mkp,
        "ident": np.eye(128, dtype=np.float32),
        "identb": np.eye(128, dtype=np.float32).astype(ml_dtypes.bfloat16),
        "indh": np.ascontiguousarray(
            np.concatenate(
                [
                    np.where(np.arange(128) < 64, 1.0, 0.0),
                    np.where(np.arange(128) >= 64, 1.0, 0.0),
                ]
            ).astype(np.float32)[None, :]
        ),
    }


def kernel(query, key_value, mask, position_bias, Wq, Wk, Wv, Wo, _trace=False):
    query = np.asarray(query, dtype=np.float32)
    key_value = np.asarray(key_value, dtype=np.float32)
    mask = np.asarray(mask)
    position_bias = np.asarray(position_bias, dtype=np.float32)
    Wq = np.asarray(Wq, dtype=np.float32)
    Wk = np.asarray(Wk, dtype=np.float32)
    Wv = np.asarray(Wv, dtype=np.float32)
    Wo = np.asarray(Wo, dtype=np.float32)

    nc = _build()
    shared = _prep_shared(query, key_value, mask)
    in_maps = [
        _prep_core(c, query, key_value, mask, position_bias, Wq, Wk, Wv, Wo, shared)
        for c in range(N_CORES)
    ]
    res = run_bass_kernel_spmd(
        nc, in_maps, list(range(N_CORES)), trace=_trace
    )
    _CACHE["last_result"] = res
    acc = res.results[0]["out"].astype(np.float64)
    for c in range(1, N_CORES):
        acc += res.results[c]["out"]
    return acc.astype(np.float32)


# revision 17
# speedup vs baseline: 1.2255x; 1.0558x over previous
"""CPM3 attention kernel for 8 trn2 NeuronCores.

Sharding: tensor-parallel over heads (2 heads/core x both batches).
Device computes per-core partial outputs (Wo row-sharded); host sums.
"""

import sys

sys.path.insert(0, "/opt/trn_rl_repo")

import numpy as np
import ml_dtypes

import concourse.bass as bass
import concourse.bacc as bacc
import concourse.tile as tile
import concourse.mybir as mybir
from concourse.bass_utils import run_bass_kernel_spmd

B, L, D, H, DH = 2, 2048, 1024, 16, 64
N_CORES = 8
HPC = H // N_CORES  # heads per core = 2
D2H = HPC * DH  # 128
QTS = 512  # q tile size
QN = L // QTS  # 4
KP = 128  # k partition tile
KN = L // KP  # 16
KTG = 4  # k tiles per DMA group
KGN = KN // KTG  # 4
DC = D // 128  # 8 contraction chunks
HVW = 2 * (DH + 1)  # 130: hv_aug columns per k-tile (2 heads x (64+ones))

F32 = mybir.dt.float32
F32R = mybir.dt.float32r
U8 = mybir.dt.uint8
BF16 = mybir.dt.bfloat16
MM_NEG = 1.0e9  # additive mask magnitude

_CACHE: dict = {}


def _build():
    if "nc" in _CACHE:
        return _CACHE["nc"]
    nc = bacc.Bacc("TRN2", target_bir_lowering=False, debug=False, num_devices=N_CORES)

    qT = nc.dram_tensor("qT", [B, DC, 128, L], F32R, kind="ExternalInput").ap()
    kvT = nc.dram_tensor("kvT", [B, DC, 128, L], F32R, kind="ExternalInput").ap()
    wq = nc.dram_tensor("wq", [128, DC, 128], F32R, kind="ExternalInput").ap()
    wk = nc.dram_tensor("wk", [128, DC, 128], F32R, kind="ExternalInput").ap()
    wv = nc.dram_tensor("wv", [128, DC, 128], F32R, kind="ExternalInput").ap()
    wo = nc.dram_tensor("wo", [128, D], F32R, kind="ExternalInput").ap()
    pb = nc.dram_tensor(
        "pb", [QN, KGN, 128, KTG, HPC, QTS], F32R, kind="ExternalInput"
    ).ap()
    mk = nc.dram_tensor("mk", [QN, 128, B, KN, QTS], U8, kind="ExternalInput").ap()
    ident = nc.dram_tensor("ident", [128, 128], F32R, kind="ExternalInput").ap()
    identb = nc.dram_tensor("identb", [128, 128], mybir.dt.bfloat16, kind="ExternalInput").ap()
    indh = nc.dram_tensor("indh", [1, 256], F32, kind="ExternalInput").ap()
    out = nc.dram_tensor("out", [B, L, D], F32, kind="ExternalOutput").ap()

    with tile.TileContext(nc) as tc:
        with (
            tc.tile_pool(name="const", bufs=1) as constp,
            tc.tile_pool(name="hq", bufs=2) as hqp,
            tc.tile_pool(name="hk", bufs=2) as hkp,
            tc.tile_pool(name="hv", bufs=2) as hvp,
            tc.tile_pool(name="big", bufs=3) as bigp,
            tc.tile_pool(name="mask", bufs=2) as mkp,
            tc.tile_pool(name="ma", bufs=4) as map_,
            tc.tile_pool(name="pt", bufs=3) as ptp,
            tc.tile_pool(name="ctxn", bufs=2) as ctxnp,
            tc.tile_pool(name="rc", bufs=2) as rcp,
            tc.tile_pool(name="outb", bufs=3) as outp,
            tc.tile_pool(name="psum", bufs=4, space=bass.MemorySpace.PSUM) as psp,
            tc.tile_pool(name="psw", bufs=2, space=bass.MemorySpace.PSUM) as psw,
        ):
            # ---- constants ----
            ident_t = constp.tile([128, 128], F32R, tag="ident")
            nc.sync.dma_start(ident_t[:], ident[:])
            identb_t = constp.tile([128, 128], BF16, tag="identb")
            nc.sync.dma_start(identb_t[:], identb[:])
            indh_t = constp.tile([1, 256], F32, tag="indh")
            nc.sync.dma_start(indh_t[:], indh[:])
            wq_t = constp.tile([128, DC, 128], F32R, tag="wq")
            nc.sync.dma_start(wq_t[:], wq[:])
            wk_t = constp.tile([128, DC, 128], F32R, tag="wk")
            nc.sync.dma_start(wk_t[:], wk[:])
            wv_t = constp.tile([128, DC, 128], F32R, tag="wv")
            nc.sync.dma_start(wv_t[:], wv[:])
            wo_t = constp.tile([128, D], F32R, tag="wo")
            nc.sync.dma_start(wo_t[:], wo[:])

            # ---- prologue: projections ----
            hq_sb, hk_sb, hv_sb = {}, {}, {}
            for b in range(B):
                hq_sb[b] = hqp.tile([128, L], F32R, tag="hq", name=f"hq_sb{b}")
                hq_ps = [psw.tile([128, 2 * QTS], F32, tag="w", name=f"hq_ps{b}_{i}") for i in range(2)]
                for dc in range(DC):
                    qc = bigp.tile([128, L], F32R, tag="big")
                    nc.sync.dma_start(qc[:], qT[b, dc])
                    for qt in range(QN):
                        nc.tensor.matmul(
                            hq_ps[qt // 2][:, (qt % 2) * QTS : (qt % 2 + 1) * QTS],
                            wq_t[:, dc, :],
                            qc[:, qt * QTS : (qt + 1) * QTS],
                            start=(dc == 0),
                            stop=(dc == DC - 1),
                        )
                for qt in range(QN):
                    nc.vector.tensor_copy(
                        hq_sb[b][:, qt * QTS : (qt + 1) * QTS],
                        hq_ps[qt // 2][:, (qt % 2) * QTS : (qt % 2 + 1) * QTS],
                    )

                hk_sb[b] = hkp.tile([128, L], F32R, tag="hk", name=f"hk_sb{b}")
                hvT = bigp.tile([128, L], F32R, tag="big")
                hk_ps = [psw.tile([128, 2 * QTS], F32, tag="w", name=f"hk_ps{b}_{i}") for i in range(2)]
                hv_ps = [psp.tile([128, QTS], F32, tag="bank", name=f"hv_ps{b}_{i}") for i in range(QN)]
                for dc in range(DC):
                    kc = bigp.tile([128, L], F32R, tag="big")
                    nc.sync.dma_start(kc[:], kvT[b, dc])
                    for qt in range(QN):
                        nc.tensor.matmul(
                            hk_ps[qt // 2][:, (qt % 2) * QTS : (qt % 2 + 1) * QTS],
                            wk_t[:, dc, :],
                            kc[:, qt * QTS : (qt + 1) * QTS],
                            start=(dc == 0),
                            stop=(dc == DC - 1),
                        )
                        nc.tensor.matmul(
                            hv_ps[qt][:],
                            wv_t[:, dc, :],
                            kc[:, qt * QTS : (qt + 1) * QTS],
                            start=(dc == 0),
                            stop=(dc == DC - 1),
                        )
                for qt in range(QN):
                    nc.vector.tensor_copy(
                        hk_sb[b][:, qt * QTS : (qt + 1) * QTS],
                        hk_ps[qt // 2][:, (qt % 2) * QTS : (qt % 2 + 1) * QTS],
                    )
                    nc.vector.tensor_copy(
                        hvT[:, qt * QTS : (qt + 1) * QTS], hv_ps[qt][:]
                    )

                # hv_aug: transpose hvT per k-tile; ones cols prefilled
                hv_sb[b] = hvp.tile([128, KN * HVW], F32R, tag="hv", name=f"hv_sb{b}")
                nc.gpsimd.memset(hv_sb[b][:].bitcast(mybir.dt.uint32), 0x3F800000)
                for kt in range(KN):
                    tp = psp.tile([128, 128], F32R, tag="bank")
                    nc.tensor.transpose(
                        tp[:], hvT[:, kt * KP : (kt + 1) * KP], ident_t[:]
                    )
                    o = kt * HVW
                    nc.vector.tensor_copy(hv_sb[b][:, o : o + DH], tp[:, 0:DH])
                    nc.vector.tensor_copy(
                        hv_sb[b][:, o + DH + 1 : o + 2 * DH + 1], tp[:, DH:128]
                    )

            # ---- main loop ----
            for qt in range(QN):
                mk_t = mkp.tile([128, B, KN, QTS], U8, tag="mask")
                nc.sync.dma_start(mk_t[:], mk[qt])
                ctx_ps = {}
                for b in range(B):
                    for h in range(HPC):
                        ctx_ps[(b, h)] = psp.tile(
                            [DH + 1, QTS], F32, tag="bank", name=f"ctx_ps{b}_{h}_{qt}"
                        )
                for kg in range(KGN):
                    pb_t = bigp.tile([128, KTG, HPC, QTS], F32R, tag="big")
                    nc.sync.dma_start(pb_t[:], pb[qt, kg])
                    for ki in range(KTG):
                        kt = kg * KTG + ki
                        ma_t = {}
                        for b in range(B):
                            ma_t[b] = map_.tile(
                                [128, QTS], BF16, tag="ma", name=f"ma_t{b}_{kt}"
                            )
                            nc.vector.tensor_scalar(
                                ma_t[b][:],
                                mk_t[:, b, kt, :],
                                MM_NEG,
                                -MM_NEG,
                                mybir.AluOpType.mult,
                                mybir.AluOpType.add,
                            )
                        # one (b, kt) pipeline unit per psw slot
                        for b in range(B):
                            sc = psw.tile(
                                [128, 2 * QTS], F32, tag="w", name=f"sc{b}_{kt}"
                            )
                            for h in range(HPC):
                                nc.tensor.matmul(
                                    sc[:, h * QTS : (h + 1) * QTS],
                                    ident_t[:],
                                    pb_t[:, ki, h, :],
                                    start=True,
                                    stop=False,
                                )
                            for h in range(HPC):
                                nc.tensor.matmul(
                                    sc[:, h * QTS : (h + 1) * QTS],
                                    identb_t[:],
                                    ma_t[b][:],
                                    start=False,
                                    stop=False,
                                )
                            # QK pair: h0 rows 0-63, h1 rows 64-127 run concurrently
                            for h in range(HPC):
                                nc.tensor.matmul(
                                    sc[:, h * QTS : (h + 1) * QTS],
                                    hk_sb[b][h * DH : (h + 1) * DH, kt * KP : (kt + 1) * KP],
                                    hq_sb[b][h * DH : (h + 1) * DH, qt * QTS : (qt + 1) * QTS],
                                    start=False,
                                    stop=True,
                                )
                            p_t = ptp.tile(
                                [128, 2 * QTS], F32R, tag="pt", name=f"p_t{b}_{kt}"
                            )
                            nc.scalar.activation(
                                p_t[:], sc[:], mybir.ActivationFunctionType.Exp
                            )
                            for h in range(HPC):
                                o = kt * HVW + h * (DH + 1)
                                nc.tensor.matmul(
                                    ctx_ps[(b, h)][:],
                                    hv_sb[b][:, o : o + DH + 1],
                                    p_t[:, h * QTS : (h + 1) * QTS],
                                    start=(kt == 0),
                                    stop=(kt == KN - 1),
                                )
                # normalize + output projection
                for b in range(B):
                    ctxn = ctxnp.tile([128, QTS], F32R, tag="ctxn")
                    bcw = psw.tile([128, 2 * QTS], F32, tag="w", name=f"bcw{b}")
                    bc = bcw[:, 0:QTS]
                    for h in range(HPC):
                        dsb = rcp.tile([1, QTS], F32, tag="dsb", name=f"dsb{b}_{h}")
                        nc.vector.tensor_copy(dsb[:], ctx_ps[(b, h)][DH : DH + 1, :])
                        rcf = rcp.tile([1, QTS], F32, tag="rcf", name=f"rcf{b}_{h}")
                        nc.vector.reciprocal_approx_fast(rcf[:], dsb[:])
                        nc.tensor.matmul(
                            bc,
                            indh_t[:, h * 128 : (h + 1) * 128],
                            rcf[:],
                            start=(h == 0),
                            stop=(h == HPC - 1),
                        )
                    bc_sb = rcp.tile([128, QTS], F32, tag="bcsb", name=f"bc_sb{b}")
                    nc.vector.tensor_copy(bc_sb[:], bc)
                    for h in range(HPC):
                        nc.vector.tensor_tensor(
                            ctxn[h * DH : (h + 1) * DH, :],
                            ctx_ps[(b, h)][0:DH, :],
                            bc_sb[h * DH : (h + 1) * DH, :],
                            mybir.AluOpType.mult,
                        )
                    for qs in range(QN):
                        ob = outp.tile([128, D], F32, tag="outb")
                        op_ps = psw.tile([128, 2 * QTS], F32, tag="w", name=f"op{b}_{qs}")
                        for oh in range(2):
                            nc.tensor.matmul(
                                op_ps[:, oh * QTS : (oh + 1) * QTS],
                                ctxn[:, qs * 128 : (qs + 1) * 128],
                                wo_t[:, oh * QTS : (oh + 1) * QTS],
                                start=True,
                                stop=True,
                            )
                        nc.vector.tensor_copy(ob[:, 0:QTS], op_ps[:, 0:QTS])
                        nc.scalar.copy(ob[:, QTS : 2 * QTS], op_ps[:, QTS : 2 * QTS])
                        r0 = qt * QTS + qs * 128
                        nc.sync.dma_start(out[b, r0 : r0 + 128, :], ob[:])

    nc.compile()
    _CACHE["nc"] = nc
    return nc


def _prep_core(core, query, key_value, mask, position_bias, Wq, Wk, Wv, Wo, shared):
    """Per-core input map. `shared` holds core-independent packed arrays."""
    h0 = core * HPC
    rows = slice(h0 * DH, (h0 + HPC) * DH)
    # weight chunks packed [dp, dc, d2h] from W[rows].T [D, 128]
    def packw(w, scale=1.0):
        wt = np.ascontiguousarray(
            (w[rows].T * scale).reshape(DC, 128, 128).transpose(1, 0, 2),
            dtype=np.float32,
        )
        return wt

    pbc = position_bias[h0 : h0 + HPC]  # [2, q, k]
    # -> [qt, kg, kp, ki, h, qf]
    pbp = np.ascontiguousarray(
        pbc.reshape(HPC, QN, QTS, KGN, KTG, 128).transpose(1, 3, 5, 4, 0, 2),
        dtype=np.float32,
    )
    m = {
        "qT": shared["qT"],
        "kvT": shared["kvT"],
        "mk": shared["mk"],
        "ident": shared["ident"],
        "identb": shared["identb"],
        "indh": shared["indh"],
        "wq": packw(Wq, 1.0 / np.sqrt(DH)),
        "wk": packw(Wk),
        "wv": packw(Wv),
        "wo": np.ascontiguousarray(Wo[:, rows].T, dtype=np.float32),
        "pb": pbp,
    }
    return m


def _prep_shared(query, key_value, mask):
    qTp = np.ascontiguousarray(
        query.reshape(B, L, DC, 128).transpose(0, 2, 3, 1), dtype=np.float32
    )
    kvTp = np.ascontiguousarray(
        key_value.reshape(B, L, DC, 128).transpose(0, 2, 3, 1), dtype=np.float32
    )
    mku = mask.astype(np.uint8)
    # [b, q, k] -> [qt, kp, b, kt, qf]
    mkp = np.ascontiguousarray(
        mku.reshape(B, QN, QTS, KN, 128).transpose(1, 4, 0, 3, 2)
    )
    return {
        "qT": qTp,
        "kvT": kvTp,
        "mk": mkp,
        "ident": np.eye(128, dtype=np.float32),
        "identb": np.eye(128, dtype=np.float32).astype(ml_dtypes.bfloat16),
        "indh": np.ascontiguousarray(
            np.concatenate(
                [
                    np.where(np.arange(128) < 64, 1.0, 0.0),
                    np.where(np.arange(128) >= 64, 1.0, 0.0),
                ]
            ).astype(np.float32)[None, :]
        ),
    }


def kernel(query, key_value, mask, position_bias, Wq, Wk, Wv, Wo, _trace=False):
    query = np.asarray(query, dtype=np.float32)
    key_value = np.asarray(key_value, dtype=np.float32)
    mask = np.asarray(mask)
    position_bias = np.asarray(position_bias, dtype=np.float32)
    Wq = np.asarray(Wq, dtype=np.float32)
    Wk = np.asarray(Wk, dtype=np.float32)
    Wv = np.asarray(Wv, dtype=np.float32)
    Wo = np.asarray(Wo, dtype=np.float32)

    nc = _build()
    shared = _prep_shared(query, key_value, mask)
    in_maps = [
        _prep_core(c, query, key_value, mask, position_bias, Wq, Wk, Wv, Wo, shared)
        for c in range(N_CORES)
    ]
    res = run_bass_kernel_spmd(
        nc, in_maps, list(range(N_CORES)), trace=_trace
    )
    _CACHE["last_result"] = res
    acc = res.results[0]["out"].astype(np.float64)
    for c in range(1, N_CORES):
        acc += res.results[c]["out"]
    return acc.astype(np.float32)
